# revision 2
# baseline (speedup 1.0000x reference)
"""Block-diagonal linear kernel for Trainium2 (8 NeuronCores, SPMD data-parallel).

Computes out = node_emb @ block_diag(blocks)^T where node_emb is [65536, 4096]
fp32 and blocks is [64, 64, 64] fp32 (64 independent 64x64 conv blocks).

Math: out[b, 128t+o] = sum_c x[b, 128t+c] * WT[t][c, o] for 32 diagonal
128x128 weight tiles WT[t] (each packing two 64x64 conv blocks on its
diagonal). Per core: 8192 rows, 64 row-tiles of 128; per row-tile the PE
transposes each 128x128 x-tile (contraction dim must sit on partitions),
then runs one 128x128x128 matmul per column tile.

Sharding: node_emb rows split 8 ways (data-parallel); the 2 MiB packed
weight tensor is replicated.

Precision: inputs/outputs are cast to fp16 on host, matmul accumulates in
fp32 PSUM. Measured end-to-end error vs the fp32 reference is ~5e-4
(scale-relative absmax). Set DT_MODE = "f32" for exact fp32 compute.
"""

import numpy as np

import concourse.bass as bass
import concourse.mybir as mybir
from concourse import bacc, tile
from concourse.bass_utils import run_bass_kernel_spmd
from concourse.masks import make_identity

N_CORES = 8
N_NODES = 65536
EMB = 4096
CONV = 64
P = 128
NT = EMB // P  # 32 column tiles
ROWS = N_NODES // N_CORES  # 8192 rows per core
F32 = mybir.dt.float32
F16 = mybir.dt.float16

DT_MODE = "f16"  # "f16" or "f32"


def build_program(rows: int = ROWS, mode: str = DT_MODE):
    dt_io = F16 if mode == "f16" else F32
    nc = bacc.Bacc(
        "TRN2", target_bir_lowering=False, debug=False, num_devices=N_CORES
    )
    x_d = nc.dram_tensor("x", [rows, EMB], dt_io, kind="ExternalInput").ap()
    w_d = nc.dram_tensor("wt", [P, NT, P], dt_io, kind="ExternalInput").ap()
    o_d = nc.dram_tensor("out", [rows, EMB], dt_io, kind="ExternalOutput").ap()
    n_bt = rows // P

    with tile.TileContext(nc) as tc:
        with (
            tc.tile_pool(name="const", bufs=1) as cpool,
            tc.tile_pool(name="xin", bufs=3) as xpool,
            tc.tile_pool(name="oout", bufs=3) as opool,
            tc.tile_pool(name="xt", bufs=6) as xtpool,
            tc.tile_pool(name="tps", bufs=3, space=bass.MemorySpace.PSUM) as tpsum,
            tc.tile_pool(name="mps", bufs=3, space=bass.MemorySpace.PSUM) as mpsum,
        ):
            ident = cpool.tile([P, P], dt_io)
            make_identity(nc, ident[:])
            w_sb = cpool.tile([P, NT, P], dt_io)
            nc.sync.dma_start(w_sb[:], w_d[:])

            for bi in range(n_bt):
                x_sb = xpool.tile([P, EMB], dt_io)
                nc.sync.dma_start(x_sb[:], x_d[bi * P : (bi + 1) * P, :])
                o_sb = opool.tile([P, EMB], dt_io)
                for g in range(NT // 4):  # 4 column tiles per PSUM bank
                    m_ps = mpsum.tile([P, 4 * P], F32)
                    for h in range(2):
                        th = 2 * g + h
                        t_ps = tpsum.tile([P, 2 * P], dt_io)
                        nc.tensor.transpose(
                            t_ps[:, 0:P],
                            x_sb[:, (2 * th) * P : (2 * th + 1) * P],
                            ident[:],
                        )
                        nc.tensor.transpose(
                            t_ps[:, P : 2 * P],
                            x_sb[:, (2 * th + 1) * P : (2 * th + 2) * P],
                            ident[:],
                        )
                        xt_sb = xtpool.tile([P, 2 * P], dt_io)
                        nc.vector.tensor_copy(xt_sb[:], t_ps[:])
                        for j in range(2):
                            t = 2 * th + j
                            nc.tensor.matmul(
                                m_ps[:, (2 * h + j) * P : (2 * h + j + 1) * P],
                                xt_sb[:, j * P : (j + 1) * P],
                                w_sb[:, t, :],
                                start=True,
                                stop=True,
                            )
                    dst = o_sb[:, g * 4 * P : (g + 1) * 4 * P]
                    if g % 2 == 0:
                        nc.scalar.copy(dst, m_ps[:])
                    else:
                        nc.vector.tensor_copy(dst, m_ps[:])
                nc.gpsimd.dma_start(o_d[bi * P : (bi + 1) * P, :], o_sb[:])

    nc.compile()
    return nc


def pack_weights(blocks: np.ndarray) -> np.ndarray:
    """Pack [64, 64, 64] conv blocks into [128(c), 32(t), 128(o)]:
    wt[c, t, o] = block_diag(blocks)[128t+o, 128t+c]."""
    bt = np.ascontiguousarray(blocks.transpose(2, 0, 1))  # [c, n, o]
    wt = np.zeros((P, NT, P), np.float32)
    wt[:CONV, :, :CONV] = bt[:, 0::2, :]
    wt[CONV:, :, CONV:] = bt[:, 1::2, :]
    return wt


_PROGRAM = None


def kernel(node_emb: np.ndarray, blocks: np.ndarray) -> np.ndarray:
    global _PROGRAM
    node_emb = np.asarray(node_emb, dtype=np.float32)
    blocks = np.asarray(blocks, dtype=np.float32)
    assert node_emb.shape == (N_NODES, EMB) and blocks.shape == (CONV, CONV, CONV)

    if _PROGRAM is None:
        _PROGRAM = build_program(ROWS, DT_MODE)
    nc = _PROGRAM

    np_dt = np.float16 if DT_MODE == "f16" else np.float32
    wt = pack_weights(blocks).astype(np_dt)
    x = node_emb.astype(np_dt) if np_dt != np.float32 else node_emb
    in_maps = [
        {"x": x[i * ROWS : (i + 1) * ROWS], "wt": wt} for i in range(N_CORES)
    ]
    res = run_bass_kernel_spmd(nc, in_maps, core_ids=list(range(N_CORES)))
    out = np.concatenate([r["out"] for r in res.results], axis=0)
    return np.ascontiguousarray(out.astype(np.float32))


# revision 5
# speedup vs baseline: 1.1934x; 1.1934x over previous
"""Block-diagonal linear kernel for Trainium2 (8 NeuronCores, SPMD data-parallel).

Computes out = node_emb @ block_diag(blocks)^T where node_emb is [65536, 4096]
fp32 and blocks is [64, 64, 64] fp32 (64 independent 64x64 conv blocks).

Math: out[b, 128t+o] = sum_c x[b, 128t+c] * WT[t][c, o] for 32 diagonal
128x128 weight tiles WT[t] (each packing two 64x64 conv blocks on its
diagonal). Per core: 8192 rows, 64 row-tiles of 128; per row-tile the PE
transposes each 128x128 x-tile (contraction dim must sit on partitions),
then runs one 128x128x128 matmul per column tile.

Sharding: node_emb rows split 8 ways (data-parallel); the 2 MiB packed
weight tensor is replicated.

Precision: inputs/outputs are cast to fp16 on host, matmul accumulates in
fp32 PSUM. Measured end-to-end error vs the fp32 reference is ~5e-4
(scale-relative absmax). Set DT_MODE = "f32" for exact fp32 compute.
"""

import numpy as np

import concourse.bass as bass
import concourse.mybir as mybir
from concourse import bacc, tile
from concourse.bass_utils import run_bass_kernel_spmd
from concourse.masks import make_identity

N_CORES = 8
N_NODES = 65536
EMB = 4096
CONV = 64
P = 128
NT = EMB // P  # 32 column tiles
ROWS = N_NODES // N_CORES  # 8192 rows per core
F32 = mybir.dt.float32
F16 = mybir.dt.float16

DT_MODE = "f16"  # "f16" or "f32"


def build_program(rows: int = ROWS, mode: str = DT_MODE, reps: int = 1):
    """reps>1 wraps the sweep in a For_i loop (timing probes only)."""
    dt_io = F16 if mode == "f16" else F32
    nc = bacc.Bacc(
        "TRN2", target_bir_lowering=False, debug=False, num_devices=N_CORES
    )
    x_d = nc.dram_tensor("x", [rows, EMB], dt_io, kind="ExternalInput").ap()
    w_d = nc.dram_tensor("wt", [P, NT, P], dt_io, kind="ExternalInput").ap()
    o_d = nc.dram_tensor("out", [rows, EMB], dt_io, kind="ExternalOutput").ap()
    n_bt = rows // P

    with tile.TileContext(nc) as tc:
        with (
            tc.tile_pool(name="const", bufs=1) as cpool,
            tc.tile_pool(name="xin", bufs=4) as xpool,
            tc.tile_pool(name="oout", bufs=4) as opool,
            tc.tile_pool(name="xt", bufs=6) as xtpool,
            tc.tile_pool(name="tps", bufs=4, space=bass.MemorySpace.PSUM) as tpsum,
            tc.tile_pool(name="mps", bufs=4, space=bass.MemorySpace.PSUM) as mpsum,
        ):
            ident = cpool.tile([P, P], dt_io)
            make_identity(nc, ident[:])
            w_sb = cpool.tile([P, NT, P], dt_io)
            nc.sync.dma_start(w_sb[:], w_d[:])

            def body():
              for bi in range(n_bt):
                x_sb = xpool.tile([P, EMB], dt_io)
                nc.sync.dma_start(x_sb[:], x_d[bi * P : (bi + 1) * P, :])
                o_sb = opool.tile([P, EMB], dt_io)
                for g in range(NT // 4):  # 4 column tiles per PSUM bank
                    m_ps = mpsum.tile([P, 4 * P], F32)
                    t_ps = tpsum.tile([P, 4 * P], dt_io)
                    for k in range(4):
                        t = 4 * g + k
                        nc.tensor.transpose(
                            t_ps[:, k * P : (k + 1) * P],
                            x_sb[:, t * P : (t + 1) * P],
                            ident[:],
                        )
                    xt_sb = xtpool.tile([P, 4 * P], dt_io)
                    nc.vector.tensor_copy(xt_sb[:], t_ps[:])
                    for k in range(4):
                        t = 4 * g + k
                        nc.tensor.matmul(
                            m_ps[:, k * P : (k + 1) * P],
                            xt_sb[:, k * P : (k + 1) * P],
                            w_sb[:, t, :],
                            start=True,
                            stop=True,
                        )
                    dst = o_sb[:, g * 4 * P : (g + 1) * 4 * P]
                    if g % 4 == 3:
                        nc.vector.tensor_copy(dst, m_ps[:])
                    else:
                        nc.scalar.copy(dst, m_ps[:])
                nc.gpsimd.dma_start(o_d[bi * P : (bi + 1) * P, :], o_sb[:])

            if reps == 1:
                body()
            else:
                with tc.For_i(0, reps, 1):
                    body()

    nc.compile()
    return nc


def pack_weights(blocks: np.ndarray) -> np.ndarray:
    """Pack [64, 64, 64] conv blocks into [128(c), 32(t), 128(o)]:
    wt[c, t, o] = block_diag(blocks)[128t+o, 128t+c]."""
    bt = np.ascontiguousarray(blocks.transpose(2, 0, 1))  # [c, n, o]
    wt = np.zeros((P, NT, P), np.float32)
    wt[:CONV, :, :CONV] = bt[:, 0::2, :]
    wt[CONV:, :, CONV:] = bt[:, 1::2, :]
    return wt


_PROGRAM = None


def kernel(node_emb: np.ndarray, blocks: np.ndarray) -> np.ndarray:
    global _PROGRAM
    node_emb = np.asarray(node_emb, dtype=np.float32)
    blocks = np.asarray(blocks, dtype=np.float32)
    assert node_emb.shape == (N_NODES, EMB) and blocks.shape == (CONV, CONV, CONV)

    if _PROGRAM is None:
        _PROGRAM = build_program(ROWS, DT_MODE)
    nc = _PROGRAM

    np_dt = np.float16 if DT_MODE == "f16" else np.float32
    wt = pack_weights(blocks).astype(np_dt)
    x = node_emb.astype(np_dt) if np_dt != np.float32 else node_emb
    in_maps = [
        {"x": x[i * ROWS : (i + 1) * ROWS], "wt": wt} for i in range(N_CORES)
    ]
    res = run_bass_kernel_spmd(nc, in_maps, core_ids=list(range(N_CORES)))
    out = np.concatenate([r["out"] for r in res.results], axis=0)
    return np.ascontiguousarray(out.astype(np.float32))


# revision 6
# speedup vs baseline: 17.7171x; 14.8455x over previous
"""Block-diagonal linear kernel for Trainium2 (8 NeuronCores, SPMD data-parallel).

Computes out = node_emb @ block_diag(blocks)^T where node_emb is [65536, 4096]
fp32 and blocks is [64, 64, 64] fp32 (64 independent 64x64 conv blocks).

Math: out[b, 128t+o] = sum_c x[b, 128t+c] * WT[t][c, o] for 32 diagonal
128x128 weight tiles WT[t] (each packing two 64x64 conv blocks on its
diagonal). Per core: 8192 rows, 64 row-tiles of 128; per row-tile the PE
transposes each 128x128 x-tile (contraction dim must sit on partitions),
then runs one 128x128x128 matmul per column tile.

Sharding: node_emb rows split 8 ways (data-parallel); the 2 MiB packed
weight tensor is replicated.

Precision: inputs/outputs are cast to fp16 on host, matmul accumulates in
fp32 PSUM. Measured end-to-end error vs the fp32 reference is ~5e-4
(scale-relative absmax). Set DT_MODE = "f32" for exact fp32 compute.
"""

import numpy as np

import concourse.bass as bass
import concourse.mybir as mybir
from concourse import bacc, tile
from concourse.bass_utils import run_bass_kernel_spmd
from concourse.masks import make_identity

N_CORES = 8
N_NODES = 65536
EMB = 4096
CONV = 64
P = 128
NT = EMB // P  # 32 column tiles
ROWS = N_NODES // N_CORES  # 8192 rows per core
F32 = mybir.dt.float32
F16 = mybir.dt.float16

DT_MODE = "f16"  # "f16" or "f32"


def build_program(rows: int = ROWS, mode: str = DT_MODE, reps: int = 1):
    """reps>1 wraps the sweep in a For_i loop (timing probes only)."""
    dt_io = F16 if mode == "f16" else F32
    nc = bacc.Bacc(
        "TRN2", target_bir_lowering=False, debug=False, num_devices=N_CORES
    )
    x_d = nc.dram_tensor("x", [rows, EMB], dt_io, kind="ExternalInput").ap()
    w_d = nc.dram_tensor("wt", [P, NT, P], dt_io, kind="ExternalInput").ap()
    o_d = nc.dram_tensor("out", [rows, EMB], dt_io, kind="ExternalOutput").ap()
    n_bt = rows // P

    with tile.TileContext(nc) as tc:
        with (
            tc.tile_pool(name="const", bufs=1) as cpool,
            tc.tile_pool(name="xin", bufs=4) as xpool,
            tc.tile_pool(name="oout", bufs=4) as opool,
            tc.tile_pool(name="xt", bufs=6) as xtpool,
            tc.tile_pool(name="tps", bufs=4, space=bass.MemorySpace.PSUM) as tpsum,
            tc.tile_pool(name="mps", bufs=4, space=bass.MemorySpace.PSUM) as mpsum,
        ):
            ident = cpool.tile([P, P], dt_io)
            make_identity(nc, ident[:])
            w_sb = cpool.tile([P, NT, P], dt_io)
            nc.sync.dma_start(w_sb[:], w_d[:])

            def body():
                for bi in range(n_bt):
                    x_sb = xpool.tile([P, EMB], dt_io)
                    nc.sync.dma_start(x_sb[:], x_d[bi * P : (bi + 1) * P, :])
                    o_sb = opool.tile([P, EMB], dt_io)
                    for g in range(NT // 4):  # 4 column tiles per PSUM bank
                        m_ps = mpsum.tile([P, 4 * P], F32)
                        t_ps = tpsum.tile([P, 4 * P], dt_io)
                        for k in range(4):
                            t = 4 * g + k
                            nc.tensor.transpose(
                                t_ps[:, k * P : (k + 1) * P],
                                x_sb[:, t * P : (t + 1) * P],
                                ident[:],
                            )
                        xt_sb = xtpool.tile([P, 4 * P], dt_io)
                        nc.vector.tensor_copy(xt_sb[:], t_ps[:])
                        for k in range(4):
                            t = 4 * g + k
                            nc.tensor.matmul(
                                m_ps[:, k * P : (k + 1) * P],
                                xt_sb[:, k * P : (k + 1) * P],
                                w_sb[:, t, :],
                                start=True,
                                stop=True,
                            )
                        dst = o_sb[:, g * 4 * P : (g + 1) * 4 * P]
                        if g % 4 == 3:
                            nc.vector.tensor_copy(dst, m_ps[:])
                        else:
                            nc.scalar.copy(dst, m_ps[:])
                    nc.gpsimd.dma_start(o_d[bi * P : (bi + 1) * P, :], o_sb[:])

            if reps == 1:
                body()
            else:
                with tc.For_i(0, reps, 1):
                    body()

    nc.compile()
    return nc


def pack_weights(blocks: np.ndarray) -> np.ndarray:
    """Pack [64, 64, 64] conv blocks into [128(c), 32(t), 128(o)]:
    wt[c, t, o] = block_diag(blocks)[128t+o, 128t+c]."""
    bt = np.ascontiguousarray(blocks.transpose(2, 0, 1))  # [c, n, o]
    wt = np.zeros((P, NT, P), np.float32)
    wt[:CONV, :, :CONV] = bt[:, 0::2, :]
    wt[CONV:, :, CONV:] = bt[:, 1::2, :]
    return wt


_PROGRAM = None


def kernel(node_emb: np.ndarray, blocks: np.ndarray) -> np.ndarray:
    global _PROGRAM
    node_emb = np.asarray(node_emb, dtype=np.float32)
    blocks = np.asarray(blocks, dtype=np.float32)
    assert node_emb.shape == (N_NODES, EMB) and blocks.shape == (CONV, CONV, CONV)

    if _PROGRAM is None:
        _PROGRAM = build_program(ROWS, DT_MODE)
    nc = _PROGRAM

    np_dt = np.float16 if DT_MODE == "f16" else np.float32
    wt = pack_weights(blocks).astype(np_dt)
    x = node_emb.astype(np_dt) if np_dt != np.float32 else node_emb
    in_maps = [
        {"x": x[i * ROWS : (i + 1) * ROWS], "wt": wt} for i in range(N_CORES)
    ]
    res = run_bass_kernel_spmd(nc, in_maps, core_ids=list(range(N_CORES)))
    out = np.concatenate([r["out"] for r in res.results], axis=0)
    return np.ascontiguousarray(out.astype(np.float32))
